# revision 21
# baseline (speedup 1.0000x reference)
"""Density-aware Chamfer loss on 8 Trainium2 NeuronCores.

Sharding: batch dim B=8, one batch element per core (data parallel).
Per core, for its [4096,3] pred/gt clouds:
  - TensorE computes pairwise squared distances as augmented K=24 bf16
    matmuls: d_ij = |a_i|^2 + |b_j|^2 - 2 a_i.b_j with coordinates split
    into bf16 hi/mid/lo triples so the product set reproduces fp32-grade
    accuracy and the self-distance cancels to exactly zero.
  - Density matrices (pred-pred, gt-gt) are symmetric: only tiles with
    strip >= block(m-tile) are computed.  ScalarE applies
    exp(-0.5 d / bw^2) with a fused free-axis sum (accum_out) giving row
    sums of computed tiles; the missing lower-triangle row sums are
    recovered as column sums of the strictly-upper tiles via ones-matmuls
    on TensorE, accumulated per strip in PSUM (4 chains per bank via
    col-group tile_position).
  - VectorE handles the chamfer mins of pred-gt: tensor_scalar
    (min with +BIG, fused min-reduce accum_out) copies each PSUM tile to
    bf16 SBUF with the row min for pred->gt, and a bf16 tensor_tensor
    min folds row tiles for the gt->pred direction (column mins).
Host gathers per-core partials and does the O(B*N) tail math.
"""

import numpy as np
import ml_dtypes
from contextlib import ExitStack

import concourse.bacc as bacc
import concourse.tile as tile
from concourse import mybir
from concourse.bass_utils import run_bass_kernel_spmd

B = 8
N = 4096
N_CORES = 8
PB = 128          # rows per m-tile (PSUM partitions)
NMT = N // PB     # 32 m-tiles
SW = 512          # strip width (one PSUM bank, matmul moving max)
NS = N // SW      # 8 strips
GW = 1024         # chamfer consumer group width (2 PSUM banks)
NG = N // GW      # 4 chamfer groups per m-tile row
K = 24            # augmented contraction depth
KP = 32           # padded partition alloc for feature arrays
EPS = 1e-6

_BF16 = ml_dtypes.bfloat16
_BIG = 3.0e38


def _split3(x):
    """x float64 -> (h, m, l) float64 arrays whose values are exactly
    representable in bf16 and sum to x within ~2^-24 relative."""
    h = x.astype(_BF16).astype(np.float64)
    r = x - h
    m = r.astype(_BF16).astype(np.float64)
    l = (r - m).astype(_BF16).astype(np.float64)
    return h, m, l


# product pairs over (hi, mid, lo) kept in the augmented matmul
_PAIRS = [(0, 0), (0, 1), (1, 0), (0, 2), (2, 0), (1, 1)]


def _features(pts):
    """pts [N,3] float64 -> (U, V) float64 arrays [K, N] of
    bf16-representable values.

    Row k of U (as lhsT) times row k of V (as rhs) summed over k gives
    the pairwise squared distance  |a_i - b_j|^2  (up to ~2^-24), with
    the diagonal of a same-cloud product exactly zero.
    """
    n = pts.shape[0]
    sp = [np.stack(_split3(pts[:, c])) for c in range(3)]  # 3 x [3, N]
    U = np.zeros((K, n))
    V = np.zeros((K, n))
    x2 = np.zeros(n)
    r = 0
    for c in range(3):
        s = sp[c]
        for iu, iv in _PAIRS:
            U[r] = s[iu]
            V[r] = -2.0 * s[iv]
            x2 += s[iu] * s[iv]
            r += 1
    h, m, l = _split3(x2)
    U[18], U[19], U[20] = h, m, l
    V[18:21] = 1.0
    U[21:24] = 1.0
    V[21], V[22], V[23] = h, m, l
    return U, V


def _combined(pred, gt):
    """Build the row-group-aligned combined lhsT/rhs arrays [96, N]:
    quadrant 0 = pred-pred, quadrant 1 = gt-gt, quadrant 2 = pred-gt."""
    U_p, V_p = _features(pred)
    U_g, V_g = _features(gt)
    n = U_p.shape[1]
    Uc = np.zeros((96, n))
    Vc = np.zeros((96, n))
    Uc[0:K] = U_p
    Vc[0:K] = V_p
    Uc[32:32 + K] = U_g
    Vc[32:32 + K] = V_g
    Uc[64:64 + K] = U_p
    Vc[64:64 + K] = V_g
    return Uc.astype(_BF16), Vc.astype(_BF16)


def _build(gamma_p, gamma_g, iters=1):
    """Build + bacc-compile the 8-core SPMD program.

    iters > 1 wraps the compute in a device-side loop (benchmarking only;
    the computation is idempotent so results are unchanged)."""
    nc = bacc.Bacc(
        "TRN2", target_bir_lowering=False, debug=False, num_devices=N_CORES
    )
    f32 = mybir.dt.float32
    bf16 = mybir.dt.bfloat16
    Exp = mybir.ActivationFunctionType.Exp
    Min = mybir.AluOpType.min

    u_in = nc.dram_tensor("u_in", [96, N], bf16, kind="ExternalInput").ap()
    v_in = nc.dram_tensor("v_in", [96, N], bf16, kind="ExternalInput").ap()

    minp_o = nc.dram_tensor("minp", [PB, NMT * NG], f32, kind="ExternalOutput").ap()
    densp_o = nc.dram_tensor("densp", [PB, NMT * NS], f32, kind="ExternalOutput").ap()
    densg_o = nc.dram_tensor("densg", [PB, NMT * NS], f32, kind="ExternalOutput").ap()
    colrun_o = nc.dram_tensor("colrun", [PB, N], bf16, kind="ExternalOutput").ap()
    # column-sum chains: rows 0..6 = pred strips 1..7, rows 7..13 = gt
    csum_o = nc.dram_tensor("csum", [1, 14 * SW], f32, kind="ExternalOutput").ap()

    with tile.TileContext(nc) as tc:
        with ExitStack() as ctx:
            feat = ctx.enter_context(tc.tile_pool(name="feat", bufs=1))
            persist = ctx.enter_context(tc.tile_pool(name="persist", bufs=1))
            scr = ctx.enter_context(tc.tile_pool(name="scr", bufs=16))
            raw_pool = ctx.enter_context(tc.tile_pool(name="rawp", bufs=2))
            den_pool = ctx.enter_context(tc.tile_pool(name="denp", bufs=2, space="PSUM"))
            pg_pool = ctx.enter_context(tc.tile_pool(name="pgp", bufs=1, space="PSUM"))
            cs_pool = ctx.enter_context(tc.tile_pool(name="csp", bufs=1, space="PSUM"))

            Uc = feat.tile([96, N], bf16)
            nc.sync.dma_start(Uc[:], u_in[:])
            Vc = feat.tile([96, N], bf16)
            nc.sync.dma_start(Vc[:], v_in[:])

            ones_t = feat.tile([PB, 1], bf16)
            nc.vector.memset(ones_t[:], 1.0)

            minp_t = persist.tile([PB, NMT * NG], f32)
            densp_t = persist.tile([PB, NMT * NS], f32)
            nc.vector.memset(densp_t[:], 0.0)
            densg_t = persist.tile([PB, NMT * NS], f32)
            nc.vector.memset(densg_t[:], 0.0)
            colrun = persist.tile([PB, N], bf16)

            # colsum chains: strips 1..3 in bank cs[mat][0] at partitions
            # 32*s, strips 4..7 in bank cs[mat][1] at partitions 32*(s-4).
            cs = {}
            for mat in ("p", "g"):
                cs[mat] = [
                    cs_pool.tile([PB, SW], f32, name=f"cs_{mat}0"),
                    cs_pool.tile([PB, SW], f32, name=f"cs_{mat}1"),
                ]

            # emission order: alternate heavy (low-block) and light
            # (high-block) m-tiles so ACT density work stays level
            mt_order = []
            for i in range(NMT // 2):
                mt_order.append(i)
                mt_order.append(NMT - 1 - i)
            pos = {mt: i for i, mt in enumerate(mt_order)}

            def chain_last(s):
                # last contributor (mt < 4s) in emission order
                return max(range(4 * s), key=lambda m: pos[m])

            def colsum(mat, mt, s, rhs):
                bank = cs[mat][0] if s < 4 else cs[mat][1]
                j = (s % 4) * 32
                nc.tensor.matmul(
                    bank[j:j + 1, :], ones_t[:, 0:1], rhs,
                    start=(mt == 0), stop=(mt == chain_last(s)),
                    tile_position=(0, j), skip_group_check=True,
                )

            for mt in mt_order:
                blk = mt // 4
                msl = slice(mt * PB, (mt + 1) * PB)

                pg_tiles = []
                for g in range(NG):
                    pg_tiles.append(
                        pg_pool.tile([PB, GW], f32, tag="pg", name=f"pg_{mt}_{g}")
                    )

                scr_tiles = []
                for s in range(NS):
                    ssl = slice(s * SW, (s + 1) * SW)
                    # chamfer pred-gt strip matmul (full matrix), row group 2
                    pg = pg_tiles[s // 2]
                    nc.tensor.matmul(
                        pg[:, (s % 2) * SW:(s % 2) * SW + SW],
                        Uc[64:64 + K, msl], Vc[64:64 + K, ssl],
                        start=True, stop=True, tile_position=(64, 0),
                    )
                    if s >= blk:
                        # density tiles (upper + diagonal only), row groups 0/1
                        dp = den_pool.tile([PB, SW], f32, tag="den")
                        nc.tensor.matmul(
                            dp[:], Uc[0:K, msl], Vc[0:K, ssl],
                            start=True, stop=True, tile_position=(0, 0),
                        )
                        dg = den_pool.tile([PB, SW], f32, tag="den")
                        nc.tensor.matmul(
                            dg[:], Uc[32:32 + K, msl], Vc[32:32 + K, ssl],
                            start=True, stop=True, tile_position=(32, 0),
                        )
                        pcol = mt * NS + s
                        ep = scr.tile([PB, SW], bf16, tag="scr")
                        nc.scalar.activation(
                            ep[:], dp[:], Exp, scale=-gamma_p,
                            accum_out=densp_t[:, pcol:pcol + 1],
                        )
                        eg = scr.tile([PB, SW], bf16, tag="scr")
                        nc.scalar.activation(
                            eg[:], dg[:], Exp, scale=-gamma_g,
                            accum_out=densg_t[:, pcol:pcol + 1],
                        )
                        if s > blk:
                            scr_tiles.append((s, ep, eg))

                    if s % 2 == 1:
                        # chamfer consumers for the completed 1024 group
                        g = s // 2
                        pg = pg_tiles[g]
                        col0 = g * GW
                        pcol = mt * NG + g
                        if mt == 0:
                            nc.vector.tensor_scalar(
                                colrun[:, col0:col0 + GW], pg[:], _BIG, None,
                                Min, Min, accum_out=minp_t[:, pcol:pcol + 1],
                            )
                        else:
                            raw = raw_pool.tile([PB, GW], bf16, tag="raw")
                            nc.vector.tensor_scalar(
                                raw[:], pg[:], _BIG, None,
                                Min, Min, accum_out=minp_t[:, pcol:pcol + 1],
                            )
                            nc.vector.tensor_tensor(
                                colrun[:, col0:col0 + GW],
                                colrun[:, col0:col0 + GW], raw[:], Min,
                            )

                # column-sum matmuls for this m-tile's strictly-upper tiles
                for s, ep, eg in scr_tiles:
                    colsum("p", mt, s, ep[:])
                    colsum("g", mt, s, eg[:])

            # copy colsum chain rows PSUM -> SBUF staging, then DMA out
            cs_stage = persist.tile([1, 14 * SW], f32)
            for i, mat in enumerate(("p", "g")):
                for s in range(1, NS):
                    bank = cs[mat][0] if s < 4 else cs[mat][1]
                    j = (s % 4) * 32
                    row = 7 * i + (s - 1)
                    nc.scalar.copy(
                        cs_stage[0:1, row * SW:(row + 1) * SW],
                        bank[j:j + 1, :],
                    )
            nc.sync.dma_start(csum_o[:], cs_stage[:])

            nc.sync.dma_start(minp_o[:], minp_t[:])
            nc.sync.dma_start(densp_o[:], densp_t[:])
            nc.sync.dma_start(densg_o[:], densg_t[:])
            nc.sync.dma_start(colrun_o[:], colrun[:])

    nc.compile()
    return nc


_CACHE = {}


def _get_program(gamma_p, gamma_g):
    key = (float(gamma_p), float(gamma_g))
    if key not in _CACHE:
        _CACHE[key] = _build(*key)
    return _CACHE[key]


def kernel(predicted, ground_truth, bandwidth_pred, bandwidth_gt, _trace=False):
    predicted = np.asarray(predicted, dtype=np.float32)
    ground_truth = np.asarray(ground_truth, dtype=np.float32)
    bw_p = float(np.asarray(bandwidth_pred))
    bw_g = float(np.asarray(bandwidth_gt))
    gamma_p = 0.5 / (bw_p * bw_p)
    gamma_g = 0.5 / (bw_g * bw_g)

    nc = _get_program(gamma_p, gamma_g)

    in_maps = []
    for b in range(B):
        Ucb, Vcb = _combined(
            predicted[b].astype(np.float64), ground_truth[b].astype(np.float64)
        )
        in_maps.append({"u_in": Ucb, "v_in": Vcb})

    res = run_bass_kernel_spmd(
        nc, in_maps, core_ids=list(range(N_CORES)), trace=_trace
    )

    total_p = 0.0
    total_g = 0.0
    for b in range(B):
        r = res.results[b]
        minp = r["minp"].reshape(PB, NMT, NG).min(axis=2).T.reshape(-1)
        ming = r["colrun"].astype(np.float32).min(axis=0)
        csum = r["csum"].reshape(14, SW)
        densp = _assemble_density(r["densp"], csum[0:7])
        densg = _assemble_density(r["densg"], csum[7:14])
        total_p += (minp.astype(np.float64) / (densp + EPS)).sum()
        total_g += (ming.astype(np.float64) / (densg + EPS)).sum()

    loss = total_p / (B * N) + total_g / (B * N)
    if _trace:
        kernel._last_results = res
    return np.float32(loss)


def _assemble_density(parts, csum):
    """parts [128, NMT*NS] f32 (valid only where s >= mt//4);
    csum [7, SW]: row s-1 = accumulated column sums of strip s."""
    parts = parts.astype(np.float64).reshape(PB, NMT, NS)
    dens = np.zeros(N)
    for mt in range(NMT):
        blk = mt // 4
        rows = parts[:, mt, blk:].sum(axis=1)  # [128]
        dens[mt * PB:(mt + 1) * PB] = rows
    for s in range(1, NS):
        dens[s * SW:(s + 1) * SW] += csum[s - 1].astype(np.float64)
    return dens / (N - 1)


# revision 22
# speedup vs baseline: 3571.6973x; 3571.6973x over previous
"""Density-aware Chamfer loss on 8 Trainium2 NeuronCores.

Sharding: batch dim B=8, one batch element per core (data parallel).
Per core, for its [4096,3] pred/gt clouds:
  - TensorE computes pairwise squared distances as augmented K=24 bf16
    matmuls: d_ij = |a_i|^2 + |b_j|^2 - 2 a_i.b_j with coordinates split
    into bf16 hi/mid/lo triples so the product set reproduces fp32-grade
    accuracy and the self-distance cancels to exactly zero.
  - Density matrices (pred-pred, gt-gt) are symmetric: only tiles with
    strip >= block(m-tile) are computed.  ScalarE applies
    exp(-0.5 d / bw^2) with a fused free-axis sum (accum_out) giving row
    sums of computed tiles; the missing lower-triangle row sums are
    recovered as column sums of the strictly-upper tiles via ones-matmuls
    on TensorE, accumulated per strip in PSUM (4 chains per bank via
    col-group tile_position).
  - VectorE handles the chamfer mins of pred-gt: tensor_scalar
    (min with +BIG, fused min-reduce accum_out) copies each PSUM tile to
    bf16 SBUF with the row min for pred->gt, and a bf16 tensor_tensor
    min folds row tiles for the gt->pred direction (column mins).
Host gathers per-core partials and does the O(B*N) tail math.
"""

import numpy as np
import ml_dtypes
from contextlib import ExitStack

import concourse.bacc as bacc
import concourse.tile as tile
from concourse import mybir
from concourse.bass_utils import run_bass_kernel_spmd

B = 8
N = 4096
N_CORES = 8
PB = 128          # rows per m-tile (PSUM partitions)
NMT = N // PB     # 32 m-tiles
SW = 512          # strip width (one PSUM bank, matmul moving max)
NS = N // SW      # 8 strips
GW = 1024         # chamfer consumer group width (2 PSUM banks)
NG = N // GW      # 4 chamfer groups per m-tile row
K = 24            # augmented contraction depth
KP = 32           # padded partition alloc for feature arrays
EPS = 1e-6

_BF16 = ml_dtypes.bfloat16
_BIG = 3.0e38


def _split3(x):
    """x float64 -> (h, m, l) float64 arrays whose values are exactly
    representable in bf16 and sum to x within ~2^-24 relative."""
    h = x.astype(_BF16).astype(np.float64)
    r = x - h
    m = r.astype(_BF16).astype(np.float64)
    l = (r - m).astype(_BF16).astype(np.float64)
    return h, m, l


# product pairs over (hi, mid, lo) kept in the augmented matmul
_PAIRS = [(0, 0), (0, 1), (1, 0), (0, 2), (2, 0), (1, 1)]


def _features(pts):
    """pts [N,3] float64 -> (U, V) float64 arrays [K, N] of
    bf16-representable values.

    Row k of U (as lhsT) times row k of V (as rhs) summed over k gives
    the pairwise squared distance  |a_i - b_j|^2  (up to ~2^-24), with
    the diagonal of a same-cloud product exactly zero.
    """
    n = pts.shape[0]
    sp = [np.stack(_split3(pts[:, c])) for c in range(3)]  # 3 x [3, N]
    U = np.zeros((K, n))
    V = np.zeros((K, n))
    x2 = np.zeros(n)
    r = 0
    for c in range(3):
        s = sp[c]
        for iu, iv in _PAIRS:
            U[r] = s[iu]
            V[r] = -2.0 * s[iv]
            x2 += s[iu] * s[iv]
            r += 1
    h, m, l = _split3(x2)
    U[18], U[19], U[20] = h, m, l
    V[18:21] = 1.0
    U[21:24] = 1.0
    V[21], V[22], V[23] = h, m, l
    return U, V


def _combined(pred, gt):
    """Build the row-group-aligned combined lhsT/rhs arrays [96, N]:
    quadrant 0 = pred-pred, quadrant 1 = gt-gt, quadrant 2 = pred-gt."""
    U_p, V_p = _features(pred)
    U_g, V_g = _features(gt)
    n = U_p.shape[1]
    Uc = np.zeros((96, n))
    Vc = np.zeros((96, n))
    Uc[0:K] = U_p
    Vc[0:K] = V_p
    Uc[32:32 + K] = U_g
    Vc[32:32 + K] = V_g
    Uc[64:64 + K] = U_p
    Vc[64:64 + K] = V_g
    return Uc.astype(_BF16), Vc.astype(_BF16)


def _build(gamma_p, gamma_g, iters=1):
    """Build + bacc-compile the 8-core SPMD program.

    iters > 1 wraps the compute in a device-side loop (benchmarking only;
    the computation is idempotent so results are unchanged)."""
    nc = bacc.Bacc(
        "TRN2", target_bir_lowering=False, debug=False, num_devices=N_CORES
    )
    f32 = mybir.dt.float32
    bf16 = mybir.dt.bfloat16
    Exp = mybir.ActivationFunctionType.Exp
    Min = mybir.AluOpType.min

    u_in = nc.dram_tensor("u_in", [96, N], bf16, kind="ExternalInput").ap()
    v_in = nc.dram_tensor("v_in", [96, N], bf16, kind="ExternalInput").ap()

    minp_o = nc.dram_tensor("minp", [PB, NMT * NG], f32, kind="ExternalOutput").ap()
    densp_o = nc.dram_tensor("densp", [PB, NMT * NS], f32, kind="ExternalOutput").ap()
    densg_o = nc.dram_tensor("densg", [PB, NMT * NS], f32, kind="ExternalOutput").ap()
    colrun_o = nc.dram_tensor("colrun", [PB, N], bf16, kind="ExternalOutput").ap()
    # column-sum chains: rows 0..6 = pred strips 1..7, rows 7..13 = gt
    csum_o = nc.dram_tensor("csum", [1, 14 * SW], f32, kind="ExternalOutput").ap()

    with tile.TileContext(nc) as tc:
        with ExitStack() as ctx:
            feat = ctx.enter_context(tc.tile_pool(name="feat", bufs=1))
            persist = ctx.enter_context(tc.tile_pool(name="persist", bufs=1))
            scr = ctx.enter_context(tc.tile_pool(name="scr", bufs=16))
            raw_pool = ctx.enter_context(tc.tile_pool(name="rawp", bufs=2))
            den_pool = ctx.enter_context(tc.tile_pool(name="denp", bufs=2, space="PSUM"))
            pg_pool = ctx.enter_context(tc.tile_pool(name="pgp", bufs=1, space="PSUM"))
            cs_pool = ctx.enter_context(tc.tile_pool(name="csp", bufs=1, space="PSUM"))

            Uc = feat.tile([96, N], bf16)
            nc.sync.dma_start(Uc[:], u_in[:])
            Vc = feat.tile([96, N], bf16)
            nc.sync.dma_start(Vc[:], v_in[:])

            ones_t = feat.tile([PB, 1], bf16)
            nc.vector.memset(ones_t[:], 1.0)

            minp_t = persist.tile([PB, NMT * NG], f32)
            densp_t = persist.tile([PB, NMT * NS], f32)
            nc.vector.memset(densp_t[:], 0.0)
            densg_t = persist.tile([PB, NMT * NS], f32)
            nc.vector.memset(densg_t[:], 0.0)
            colrun = persist.tile([PB, N], bf16)

            # colsum chains: strips 1..3 in bank cs[mat][0] at partitions
            # 32*s, strips 4..7 in bank cs[mat][1] at partitions 32*(s-4).
            cs = {}
            for mat in ("p", "g"):
                cs[mat] = [
                    cs_pool.tile([PB, SW], f32, name=f"cs_{mat}0"),
                    cs_pool.tile([PB, SW], f32, name=f"cs_{mat}1"),
                ]

            # emission order: alternate heavy (low-block) and light
            # (high-block) m-tiles so ACT density work stays level
            mt_order = []
            for i in range(NMT // 2):
                mt_order.append(i)
                mt_order.append(NMT - 1 - i)
            pos = {mt: i for i, mt in enumerate(mt_order)}

            def chain_last(s):
                # last contributor (mt < 4s) in emission order
                return max(range(4 * s), key=lambda m: pos[m])

            def colsum(mat, mt, s, rhs):
                bank = cs[mat][0] if s < 4 else cs[mat][1]
                j = (s % 4) * 32
                nc.tensor.matmul(
                    bank[j:j + 1, :], ones_t[:, 0:1], rhs,
                    start=(mt == 0), stop=(mt == chain_last(s)),
                    tile_position=(0, j), skip_group_check=True,
                )

            def emit_body():
              for mt in mt_order:
                blk = mt // 4
                msl = slice(mt * PB, (mt + 1) * PB)

                pg_tiles = []
                for g in range(NG):
                    pg_tiles.append(
                        pg_pool.tile([PB, GW], f32, tag="pg", name=f"pg_{mt}_{g}")
                    )

                scr_tiles = []
                for s in range(NS):
                    ssl = slice(s * SW, (s + 1) * SW)
                    # chamfer pred-gt strip matmul (full matrix), row group 2
                    pg = pg_tiles[s // 2]
                    nc.tensor.matmul(
                        pg[:, (s % 2) * SW:(s % 2) * SW + SW],
                        Uc[64:64 + K, msl], Vc[64:64 + K, ssl],
                        start=True, stop=True, tile_position=(64, 0),
                    )
                    if s >= blk:
                        # density tiles (upper + diagonal only), row groups 0/1
                        dp = den_pool.tile([PB, SW], f32, tag="den")
                        nc.tensor.matmul(
                            dp[:], Uc[0:K, msl], Vc[0:K, ssl],
                            start=True, stop=True, tile_position=(0, 0),
                        )
                        dg = den_pool.tile([PB, SW], f32, tag="den")
                        nc.tensor.matmul(
                            dg[:], Uc[32:32 + K, msl], Vc[32:32 + K, ssl],
                            start=True, stop=True, tile_position=(32, 0),
                        )
                        pcol = mt * NS + s
                        ep = scr.tile([PB, SW], bf16, tag="scr")
                        nc.scalar.activation(
                            ep[:], dp[:], Exp, scale=-gamma_p,
                            accum_out=densp_t[:, pcol:pcol + 1],
                        )
                        eg = scr.tile([PB, SW], bf16, tag="scr")
                        nc.scalar.activation(
                            eg[:], dg[:], Exp, scale=-gamma_g,
                            accum_out=densg_t[:, pcol:pcol + 1],
                        )
                        if s > blk:
                            scr_tiles.append((s, ep, eg))

                    if s % 2 == 1:
                        # chamfer consumers for the completed 1024 group
                        g = s // 2
                        pg = pg_tiles[g]
                        col0 = g * GW
                        pcol = mt * NG + g
                        if mt == 0:
                            nc.vector.tensor_scalar(
                                colrun[:, col0:col0 + GW], pg[:], _BIG, None,
                                Min, Min, accum_out=minp_t[:, pcol:pcol + 1],
                            )
                        else:
                            raw = raw_pool.tile([PB, GW], bf16, tag="raw")
                            nc.vector.tensor_scalar(
                                raw[:], pg[:], _BIG, None,
                                Min, Min, accum_out=minp_t[:, pcol:pcol + 1],
                            )
                            nc.vector.tensor_tensor(
                                colrun[:, col0:col0 + GW],
                                colrun[:, col0:col0 + GW], raw[:], Min,
                            )

                # column-sum matmuls for this m-tile's strictly-upper tiles
                for s, ep, eg in scr_tiles:
                    colsum("p", mt, s, ep[:])
                    colsum("g", mt, s, eg[:])

            if iters > 1:
                with tc.For_i(0, iters, 1):
                    emit_body()
            else:
                emit_body()

            # copy colsum chain rows PSUM -> SBUF staging, then DMA out
            cs_stage = persist.tile([1, 14 * SW], f32)
            for i, mat in enumerate(("p", "g")):
                for s in range(1, NS):
                    bank = cs[mat][0] if s < 4 else cs[mat][1]
                    j = (s % 4) * 32
                    row = 7 * i + (s - 1)
                    nc.scalar.copy(
                        cs_stage[0:1, row * SW:(row + 1) * SW],
                        bank[j:j + 1, :],
                    )
            nc.sync.dma_start(csum_o[:], cs_stage[:])

            nc.sync.dma_start(minp_o[:], minp_t[:])
            nc.sync.dma_start(densp_o[:], densp_t[:])
            nc.sync.dma_start(densg_o[:], densg_t[:])
            nc.sync.dma_start(colrun_o[:], colrun[:])

    nc.compile()
    return nc


_CACHE = {}


def _get_program(gamma_p, gamma_g):
    key = (float(gamma_p), float(gamma_g))
    if key not in _CACHE:
        _CACHE[key] = _build(*key)
    return _CACHE[key]


def kernel(predicted, ground_truth, bandwidth_pred, bandwidth_gt, _trace=False):
    predicted = np.asarray(predicted, dtype=np.float32)
    ground_truth = np.asarray(ground_truth, dtype=np.float32)
    bw_p = float(np.asarray(bandwidth_pred))
    bw_g = float(np.asarray(bandwidth_gt))
    gamma_p = 0.5 / (bw_p * bw_p)
    gamma_g = 0.5 / (bw_g * bw_g)

    nc = _get_program(gamma_p, gamma_g)

    in_maps = []
    for b in range(B):
        Ucb, Vcb = _combined(
            predicted[b].astype(np.float64), ground_truth[b].astype(np.float64)
        )
        in_maps.append({"u_in": Ucb, "v_in": Vcb})

    res = run_bass_kernel_spmd(
        nc, in_maps, core_ids=list(range(N_CORES)), trace=_trace
    )

    total_p = 0.0
    total_g = 0.0
    for b in range(B):
        r = res.results[b]
        minp = r["minp"].reshape(PB, NMT, NG).min(axis=2).T.reshape(-1)
        ming = r["colrun"].astype(np.float32).min(axis=0)
        csum = r["csum"].reshape(14, SW)
        densp = _assemble_density(r["densp"], csum[0:7])
        densg = _assemble_density(r["densg"], csum[7:14])
        total_p += (minp.astype(np.float64) / (densp + EPS)).sum()
        total_g += (ming.astype(np.float64) / (densg + EPS)).sum()

    loss = total_p / (B * N) + total_g / (B * N)
    if _trace:
        kernel._last_results = res
    return np.float32(loss)


def _assemble_density(parts, csum):
    """parts [128, NMT*NS] f32 (valid only where s >= mt//4);
    csum [7, SW]: row s-1 = accumulated column sums of strip s."""
    parts = parts.astype(np.float64).reshape(PB, NMT, NS)
    dens = np.zeros(N)
    for mt in range(NMT):
        blk = mt // 4
        rows = parts[:, mt, blk:].sum(axis=1)  # [128]
        dens[mt * PB:(mt + 1) * PB] = rows
    for s in range(1, NS):
        dens[s * SW:(s + 1) * SW] += csum[s - 1].astype(np.float64)
    return dens / (N - 1)


# revision 26
# speedup vs baseline: 3607.0918x; 1.0099x over previous
"""Density-aware Chamfer loss on 8 Trainium2 NeuronCores.

Sharding: batch dim B=8, one batch element per core (data parallel).
Per core, for its [4096,3] pred/gt clouds:
  - TensorE computes pairwise squared distances as augmented K=24 bf16
    matmuls: d_ij = |a_i|^2 + |b_j|^2 - 2 a_i.b_j with coordinates split
    into bf16 hi/mid/lo triples so the product set reproduces fp32-grade
    accuracy and the self-distance cancels to exactly zero.
  - Density matrices (pred-pred, gt-gt) are symmetric: only tiles with
    strip >= block(m-tile) are computed.  ScalarE applies
    exp(-0.5 d / bw^2) with a fused free-axis sum (accum_out) giving row
    sums of computed tiles; the missing lower-triangle row sums are
    recovered as column sums of the strictly-upper tiles via ones-matmuls
    on TensorE, accumulated per strip in PSUM (4 chains per bank via
    col-group tile_position).
  - VectorE handles the chamfer mins of pred-gt: tensor_scalar
    (min with +BIG, fused min-reduce accum_out) copies each PSUM tile to
    bf16 SBUF with the row min for pred->gt, and a bf16 tensor_tensor
    min folds row tiles for the gt->pred direction (column mins).
Host gathers per-core partials and does the O(B*N) tail math.
"""

import numpy as np
import ml_dtypes
from contextlib import ExitStack

import concourse.bacc as bacc
import concourse.tile as tile
from concourse import mybir
from concourse.bass_utils import run_bass_kernel_spmd

B = 8
N = 4096
N_CORES = 8
PB = 128          # rows per m-tile (PSUM partitions)
NMT = N // PB     # 32 m-tiles
SW = 512          # strip width (one PSUM bank, matmul moving max)
NS = N // SW      # 8 strips
GW = 1024         # chamfer consumer group width (2 PSUM banks)
NG = N // GW      # 4 chamfer groups per m-tile row
K = 24            # augmented contraction depth
KP = 32           # padded partition alloc for feature arrays
EPS = 1e-6

_BF16 = ml_dtypes.bfloat16
_BIG = 3.0e38


def _split3(x):
    """x float64 -> (h, m, l) float64 arrays whose values are exactly
    representable in bf16 and sum to x within ~2^-24 relative."""
    h = x.astype(_BF16).astype(np.float64)
    r = x - h
    m = r.astype(_BF16).astype(np.float64)
    l = (r - m).astype(_BF16).astype(np.float64)
    return h, m, l


# product pairs over (hi, mid, lo) kept in the augmented matmul
_PAIRS = [(0, 0), (0, 1), (1, 0), (0, 2), (2, 0), (1, 1)]


def _features(pts):
    """pts [N,3] float64 -> (U, V) float64 arrays [K, N] of
    bf16-representable values.

    Row k of U (as lhsT) times row k of V (as rhs) summed over k gives
    the pairwise squared distance  |a_i - b_j|^2  (up to ~2^-24), with
    the diagonal of a same-cloud product exactly zero.
    """
    n = pts.shape[0]
    sp = [np.stack(_split3(pts[:, c])) for c in range(3)]  # 3 x [3, N]
    U = np.zeros((K, n))
    V = np.zeros((K, n))
    x2 = np.zeros(n)
    r = 0
    for c in range(3):
        s = sp[c]
        for iu, iv in _PAIRS:
            U[r] = s[iu]
            V[r] = -2.0 * s[iv]
            x2 += s[iu] * s[iv]
            r += 1
    h, m, l = _split3(x2)
    U[18], U[19], U[20] = h, m, l
    V[18:21] = 1.0
    U[21:24] = 1.0
    V[21], V[22], V[23] = h, m, l
    return U, V


def _combined(pred, gt):
    """Build the row-group-aligned combined lhsT/rhs arrays [96, N]:
    quadrant 0 = pred-pred, quadrant 1 = gt-gt, quadrant 2 = pred-gt."""
    U_p, V_p = _features(pred)
    U_g, V_g = _features(gt)
    n = U_p.shape[1]
    Uc = np.zeros((96, n))
    Vc = np.zeros((96, n))
    Uc[0:K] = U_p
    Vc[0:K] = V_p
    Uc[32:32 + K] = U_g
    Vc[32:32 + K] = V_g
    Uc[64:64 + K] = U_p
    Vc[64:64 + K] = V_g
    return Uc.astype(_BF16), Vc.astype(_BF16)


def _build(gamma_p, gamma_g, iters=1):
    """Build + bacc-compile the 8-core SPMD program.

    iters > 1 wraps the compute in a device-side loop (benchmarking only;
    the computation is idempotent so results are unchanged)."""
    nc = bacc.Bacc(
        "TRN2", target_bir_lowering=False, debug=False, num_devices=N_CORES
    )
    f32 = mybir.dt.float32
    bf16 = mybir.dt.bfloat16
    Exp = mybir.ActivationFunctionType.Exp
    Min = mybir.AluOpType.min

    u_in = nc.dram_tensor("u_in", [96, N], bf16, kind="ExternalInput").ap()
    v_in = nc.dram_tensor("v_in", [96, N], bf16, kind="ExternalInput").ap()

    minp_o = nc.dram_tensor("minp", [PB, NMT * NG], f32, kind="ExternalOutput").ap()
    densp_o = nc.dram_tensor("densp", [PB, NMT * NS], f32, kind="ExternalOutput").ap()
    densg_o = nc.dram_tensor("densg", [PB, NMT * NS], f32, kind="ExternalOutput").ap()
    colrun_o = nc.dram_tensor("colrun", [PB, N], bf16, kind="ExternalOutput").ap()
    # column-sum chains: rows 0..6 = pred strips 1..7, rows 7..13 = gt
    csum_o = nc.dram_tensor("csum", [1, 14 * SW], f32, kind="ExternalOutput").ap()

    with tile.TileContext(nc) as tc:
        with ExitStack() as ctx:
            feat = ctx.enter_context(tc.tile_pool(name="feat", bufs=1))
            persist = ctx.enter_context(tc.tile_pool(name="persist", bufs=1))
            scr = ctx.enter_context(tc.tile_pool(name="scr", bufs=16))
            raw_pool = ctx.enter_context(tc.tile_pool(name="rawp", bufs=2))
            den_pool = ctx.enter_context(tc.tile_pool(name="denp", bufs=2, space="PSUM"))
            pg_pool = ctx.enter_context(tc.tile_pool(name="pgp", bufs=1, space="PSUM"))
            cs_pool = ctx.enter_context(tc.tile_pool(name="csp", bufs=1, space="PSUM"))

            Uc = feat.tile([96, N], bf16)
            Vc = feat.tile([96, N], bf16)
            for c0 in range(0, N, GW):
                nc.sync.dma_start(Vc[:, c0:c0 + GW], v_in[:, c0:c0 + GW])
            for c0 in range(0, N, GW):
                nc.sync.dma_start(Uc[:, c0:c0 + GW], u_in[:, c0:c0 + GW])

            ones_t = feat.tile([PB, 1], bf16)
            nc.vector.memset(ones_t[:], 1.0)

            minp_t = persist.tile([PB, NMT * NG], f32)
            densp_t = persist.tile([PB, NMT * NS], f32)
            nc.vector.memset(densp_t[:], 0.0)
            densg_t = persist.tile([PB, NMT * NS], f32)
            nc.vector.memset(densg_t[:], 0.0)
            colrun = persist.tile([PB, N], bf16)

            # colsum chains: strips 1..3 in bank cs[mat][0] at partitions
            # 32*s, strips 4..7 in bank cs[mat][1] at partitions 32*(s-4).
            cs = {}
            for mat in ("p", "g"):
                cs[mat] = [
                    cs_pool.tile([PB, SW], f32, name=f"cs_{mat}0"),
                    cs_pool.tile([PB, SW], f32, name=f"cs_{mat}1"),
                ]

            # emission order: alternate heavy (low-block) and light
            # (high-block) m-tiles so ACT density work stays level
            mt_order = []
            for i in range(NMT // 2):
                mt_order.append(i)
                mt_order.append(NMT - 1 - i)
            pos = {mt: i for i, mt in enumerate(mt_order)}

            def chain_last(s):
                # last contributor (mt < 4s) in emission order
                return max(range(4 * s), key=lambda m: pos[m])

            cs_stage = persist.tile([1, 14 * SW], f32)

            def colsum(mat, mt, s, rhs):
                bank = cs[mat][0] if s < 4 else cs[mat][1]
                j = (s % 4) * 32
                nc.tensor.matmul(
                    bank[j:j + 1, :], ones_t[:, 0:1], rhs,
                    start=(mt == 0), stop=(mt == chain_last(s)),
                    tile_position=(0, j), skip_group_check=True,
                )
                if mt == chain_last(s):
                    row = 7 * (0 if mat == "p" else 1) + (s - 1)
                    nc.scalar.copy(
                        cs_stage[0:1, row * SW:(row + 1) * SW],
                        bank[j:j + 1, :],
                    )

            def emit_body():
              for mt in mt_order:
                blk = mt // 4
                msl = slice(mt * PB, (mt + 1) * PB)

                pg_tiles = []
                for g in range(NG):
                    pg_tiles.append(
                        pg_pool.tile([PB, GW], f32, tag="pg", name=f"pg_{mt}_{g}")
                    )

                scr_tiles = []
                for s in range(NS):
                    ssl = slice(s * SW, (s + 1) * SW)
                    # chamfer pred-gt strip matmul (full matrix), row group 2
                    pg = pg_tiles[s // 2]
                    nc.tensor.matmul(
                        pg[:, (s % 2) * SW:(s % 2) * SW + SW],
                        Uc[64:64 + K, msl], Vc[64:64 + K, ssl],
                        start=True, stop=True, tile_position=(64, 0),
                    )
                    if s >= blk:
                        # density tiles (upper + diagonal only), row groups 0/1
                        dp = den_pool.tile([PB, SW], f32, tag="den")
                        nc.tensor.matmul(
                            dp[:], Uc[0:K, msl], Vc[0:K, ssl],
                            start=True, stop=True, tile_position=(0, 0),
                        )
                        dg = den_pool.tile([PB, SW], f32, tag="den")
                        nc.tensor.matmul(
                            dg[:], Uc[32:32 + K, msl], Vc[32:32 + K, ssl],
                            start=True, stop=True, tile_position=(32, 0),
                        )
                        pcol = mt * NS + s
                        ep = scr.tile([PB, SW], bf16, tag="scr")
                        nc.scalar.activation(
                            ep[:], dp[:], Exp, scale=-gamma_p,
                            accum_out=densp_t[:, pcol:pcol + 1],
                        )
                        eg = scr.tile([PB, SW], bf16, tag="scr")
                        nc.scalar.activation(
                            eg[:], dg[:], Exp, scale=-gamma_g,
                            accum_out=densg_t[:, pcol:pcol + 1],
                        )
                        if s > blk:
                            scr_tiles.append((s, ep, eg))

                    if s % 2 == 1:
                        # chamfer consumers for the completed 1024 group
                        g = s // 2
                        pg = pg_tiles[g]
                        col0 = g * GW
                        pcol = mt * NG + g
                        if mt == 0:
                            nc.vector.tensor_scalar(
                                colrun[:, col0:col0 + GW], pg[:], _BIG, None,
                                Min, Min, accum_out=minp_t[:, pcol:pcol + 1],
                            )
                        else:
                            raw = raw_pool.tile([PB, GW], bf16, tag="raw")
                            nc.vector.tensor_scalar(
                                raw[:], pg[:], _BIG, None,
                                Min, Min, accum_out=minp_t[:, pcol:pcol + 1],
                            )
                            nc.vector.tensor_tensor(
                                colrun[:, col0:col0 + GW],
                                colrun[:, col0:col0 + GW], raw[:], Min,
                            )
                        if mt == mt_order[-1]:
                            nc.sync.dma_start(
                                colrun_o[:, col0:col0 + GW],
                                colrun[:, col0:col0 + GW],
                            )

                # column-sum matmuls for this m-tile's strictly-upper tiles
                for s, ep, eg in scr_tiles:
                    colsum("p", mt, s, ep[:])
                    colsum("g", mt, s, eg[:])

            if iters > 1:
                with tc.For_i(0, iters, 1):
                    emit_body()
            else:
                emit_body()

            nc.sync.dma_start(csum_o[:], cs_stage[:])
            nc.sync.dma_start(minp_o[:], minp_t[:])
            nc.sync.dma_start(densp_o[:], densp_t[:])
            nc.sync.dma_start(densg_o[:], densg_t[:])

    nc.compile()
    return nc


_CACHE = {}


def _get_program(gamma_p, gamma_g):
    key = (float(gamma_p), float(gamma_g))
    if key not in _CACHE:
        _CACHE[key] = _build(*key)
    return _CACHE[key]


def kernel(predicted, ground_truth, bandwidth_pred, bandwidth_gt, _trace=False):
    predicted = np.asarray(predicted, dtype=np.float32)
    ground_truth = np.asarray(ground_truth, dtype=np.float32)
    bw_p = float(np.asarray(bandwidth_pred))
    bw_g = float(np.asarray(bandwidth_gt))
    gamma_p = 0.5 / (bw_p * bw_p)
    gamma_g = 0.5 / (bw_g * bw_g)

    nc = _get_program(gamma_p, gamma_g)

    in_maps = []
    for b in range(B):
        Ucb, Vcb = _combined(
            predicted[b].astype(np.float64), ground_truth[b].astype(np.float64)
        )
        in_maps.append({"u_in": Ucb, "v_in": Vcb})

    res = run_bass_kernel_spmd(
        nc, in_maps, core_ids=list(range(N_CORES)), trace=_trace
    )

    total_p = 0.0
    total_g = 0.0
    for b in range(B):
        r = res.results[b]
        minp = r["minp"].reshape(PB, NMT, NG).min(axis=2).T.reshape(-1)
        ming = r["colrun"].astype(np.float32).min(axis=0)
        csum = r["csum"].reshape(14, SW)
        densp = _assemble_density(r["densp"], csum[0:7])
        densg = _assemble_density(r["densg"], csum[7:14])
        total_p += (minp.astype(np.float64) / (densp + EPS)).sum()
        total_g += (ming.astype(np.float64) / (densg + EPS)).sum()

    loss = total_p / (B * N) + total_g / (B * N)
    if _trace:
        kernel._last_results = res
    return np.float32(loss)


def _assemble_density(parts, csum):
    """parts [128, NMT*NS] f32 (valid only where s >= mt//4);
    csum [7, SW]: row s-1 = accumulated column sums of strip s."""
    parts = parts.astype(np.float64).reshape(PB, NMT, NS)
    dens = np.zeros(N)
    for mt in range(NMT):
        blk = mt // 4
        rows = parts[:, mt, blk:].sum(axis=1)  # [128]
        dens[mt * PB:(mt + 1) * PB] = rows
    for s in range(1, NS):
        dens[s * SW:(s + 1) * SW] += csum[s - 1].astype(np.float64)
    return dens / (N - 1)
